# revision 14
# baseline (speedup 1.0000x reference)
"""Trainium2 Bass kernel for nn_AttentionLayer (GAT-style masked attention).

Computes, for full inputs:
    h1 = emb_src @ W                      [8000, 128]
    g  = emb_dest @ (W @ W2)              [10000, 128]
    e  = g @ h1.T                         [10000, 8000]
    s  = lrelu(e, 0.2) * (1/sqrt(128))    masked to -inf where bias <= 0
    att = softmax(s, axis=1)
    out = att @ ft                        [10000, 1]   (ft = nan-cleaned feature_src)

Sharding: N_dest split across 8 NeuronCores (1250 rows each); emb_src /
feature_src / W / W2 replicated. No collectives. Softmax is unnormalized
(numer/denom) — no max subtraction needed since |scale*lrelu(e)| <= ~15.

Layout: TRANSPOSED on-device — scores are computed as e.T tiles
[src=partition, dest=free] so that BOTH softmax reductions (denominator
sum(u) and numerator sum(u*ft)) run on the Tensor engine as accumulating
matmuls with lhsT = [ones | ft_chunk].

v2 elementwise chain (vs v1): the bias mask is fused into the single
PSUM-consuming DVE pass — sm = min(BIG*biasT, psE) via scalar_tensor_tensor
(masked entries -> -inf fp16, kept -> psE). This deletes v1's separate ACT
mask-gen Relu pass entirely. LeakyReLU is then y = 0.2*sm (on GPSIMD, where
one-input tensor_scalar is cheap) and t = max(sm, y) column-split between
DVE (2x mode) and GPSIMD, tuned so DVE ~= GPSIMD ~= ACT(exp) per tile.

Dtype staging (host is layout/dtype only): bias is staged transposed
tile-contiguous [63,128,1280] as float8_e5m2 (sign-exact down to 2^-17;
only the sign is consumed on device) — 4x less HBM than f32. Embeddings
and weights are staged bf16 (the matmuls already ran at bf16 precision
in v1; this halves the 8MB src stream and runs preamble matmuls at
1 cyc/row instead of fp32r's 4).

Per-core device pipeline, per src tile s (63 tiles):
    DMA:   btT    = bias.T tile (fp8e5)      [128,1250]
    PE:    psE    = h1T_s.T @ gts            f32 PSUM (3 bank chunks)
    DVE:   sm     = (BIG * btT) min psE      fp16 (stt, 1x: the only PSUM pass)
    GPS:   y      = 0.2 * sm                 fp16
    DVE:   t[:, :D]  = max(sm, y)            (2x mode)
    GPS:   t[:, D:]  = max(sm, y)
    ACT:   u      = Exp(t) -> bf16           (exp(-inf) == 0)
    PE:    psR   += [ones | ft_s].T @ u      (emitted one tile LATE so the
                                             in-order PE queue never stalls
                                             the next e-matmul behind it)
Final: out = psR[num] / psR[den] per dest column, three row DMAs.
"""
import os
import sys

sys.path.insert(0, "/opt/trn_rl_repo")

import numpy as np

_CACHE = {}

N_DEST, N_SRC, IN_DIM, HID = 10000, 8000, 256, 128
N_CORES = 8
ND = N_DEST // N_CORES            # 1250 dest rows per core
NDP = 1250                        # dest width (free axis of transposed tiles)
NSP = 8064                        # src padded to 63 full 128-row tiles
NST = NSP // 128                  # 63 src tiles
SCALE = float(1.0 / np.sqrt(np.float32(HID)))

HC = 1000                         # h1T build chunk width
N_HC = N_SRC // HC                # 8

CHK = [(0, 512), (512, 512), (1024, NDP - 1024)]   # dest chunks (PSUM banks)
BIG = 1e30                        # mask scale: min(BIG*b, e) = e keep / -inf mask

D_DVE = 465                       # lrelu columns on DVE (mul+max); rest on ACT Prelu


def _build_nc():
    import concourse.bass as bass
    import concourse.tile as tile
    from concourse import bacc, mybir
    from contextlib import ExitStack

    F32 = mybir.dt.float32
    BF16 = mybir.dt.bfloat16
    FP16 = mybir.dt.float16
    F8E5 = mybir.dt.float8e5
    AF = mybir.ActivationFunctionType
    OP = mybir.AluOpType

    nc = bacc.Bacc("TRN2", target_bir_lowering=False, debug=False,
                   num_devices=N_CORES)

    bias_t = nc.declare_dram_parameter("biasT", [NST, 128, NDP], F8E5,
                                       isOutput=False)
    destT_t = nc.declare_dram_parameter("emb_destT", [IN_DIM, ND], BF16,
                                        isOutput=False)
    srcT_t = nc.declare_dram_parameter("emb_srcT", [IN_DIM, N_SRC], BF16,
                                       isOutput=False)
    ftc_t = nc.declare_dram_parameter("ft_cols", [128, NST], F32,
                                      isOutput=False)
    w_t = nc.declare_dram_parameter("W", [IN_DIM, HID], BF16, isOutput=False)
    wt_t = nc.declare_dram_parameter("WT", [HID, IN_DIM], BF16, isOutput=False)
    w2_t = nc.declare_dram_parameter("W2", [HID, HID], BF16, isOutput=False)
    out_t = nc.declare_dram_parameter("out", [1, ND], F32, isOutput=True)

    with tile.TileContext(nc) as tc, ExitStack() as ctx:
        persist = ctx.enter_context(tc.tile_pool(name="persist", bufs=1))

        # persistent tiles
        gts = persist.tile([128, NDP], BF16)      # SCALE * g.T  [hid, dest]
        h1t = persist.tile([128, NSP], BF16)      # h1.T         [hid, src]
        ftw = persist.tile([128, 2 * NST], BF16)  # per-src-tile [ones | ft]

        # ================= main loop (pools concurrent with preamble:
        # no close barrier; main tiles start as soon as deps are ready)
        pbias = ctx.enter_context(tc.tile_pool(name="mn_bias", bufs=6))
        pt = ctx.enter_context(tc.tile_pool(name="mn_t", bufs=3))
        pu = ctx.enter_context(tc.tile_pool(name="mn_u", bufs=3))
        psm = ctx.enter_context(tc.tile_pool(name="mn_small", bufs=1))
        pacc = ctx.enter_context(
            tc.tile_pool(name="mn_acc", bufs=1, space="PSUM"))
        mps = ctx.enter_context(
            tc.tile_pool(name="mn_ps", bufs=2, space="PSUM"))
        pre = ctx.enter_context(tc.tile_pool(name="pre_sb", bufs=2))
        preb = ctx.enter_context(tc.tile_pool(name="pre_big", bufs=2))
        pps = ctx.enter_context(
            tc.tile_pool(name="pre_ps", bufs=1, space="PSUM"))

        # ---- W chunks ([K=in_sub, M=hid]), bf16 (critical path: wc -> gts)
        w_sb = pre.tile([128, 2, HID], BF16, tag="w_sb")
        for c in range(2):
            nc.sync.dma_start(out=w_sb[:, c, :],
                              in_=w_t[128 * c:128 * (c + 1), :])
        w2_sb = pre.tile([128, HID], BF16, tag="w2_sb")
        nc.sync.dma_start(out=w2_sb, in_=w2_t[:, :])
        wt_sb = pre.tile([128, IN_DIM], BF16, tag="wt_sb")
        nc.sync.dma_start(out=wt_sb, in_=wt_t[:, :])

        # ---- ftw: per src tile s, columns [2s, 2s+1] = [ones, ft_s]
        ftc_sb = pre.tile([128, NST], F32, tag="ftc_sb")
        nc.sync.dma_start(out=ftc_sb, in_=ftc_t[:, :])
        ftw_v = ftw[:, :].rearrange("p (s two) -> p s two", two=2)
        nc.gpsimd.memset(ftw_v[:, :, 0], 1.0)
        nc.vector.tensor_copy(out=ftw_v[:, :, 1], in_=ftc_sb)

        # ---- Wc = W @ W2 as [K=in_sub, M=hid] chunks (lhsT = staged W.T)
        wc_sb = pre.tile([128, 2, HID], BF16, tag="wc_sb")
        for c in range(2):
            ps_mm = pps.tile([128, 512], F32, tag="ps_b")
            nc.tensor.matmul(ps_mm[:, :HID],
                             wt_sb[:, 128 * c:128 * (c + 1)], w2_sb,
                             start=True, stop=True)
            nc.scalar.copy(out=wc_sb[:, c, :], in_=ps_mm[:, :HID])

        # ---- bias tile DMA issue (sync queue, 6-buffer ring)
        bt_tiles = {}

        def issue_bt(s):
            btT = pbias.tile([128, NDP], F8E5, tag="btT", name=f"bt{s % 6}")
            nc.sync.dma_start(out=btT, in_=bias_t[s])
            bt_tiles[s] = btT

        issue_bt(0)
        issue_bt(1)

        # ---- emb_destT -> gts (= SCALE * Wc.T @ emb_dest.T)
        # dest/src preamble streams ride the ACT DMA queue so they don't
        # serialize behind the bias tiles on the sync queue
        dsb = preb.tile([128, 2, ND], BF16, tag="dsb")
        for c in range(2):
            nc.scalar.dma_start(out=dsb[:, c, :],
                                in_=destT_t[128 * c:128 * (c + 1), :])
        for d0 in range(0, NDP, 512):
            dw = min(512, NDP - d0)
            ps_g = pps.tile([128, 512], F32, tag="ps_b")
            for c in range(2):
                nc.tensor.matmul(ps_g[:, :dw],
                                 wc_sb[:, c, :],
                                 dsb[:, c, d0:d0 + dw],
                                 start=(c == 0), stop=(c == 1))
            nc.scalar.activation(out=gts[:, d0:d0 + dw], in_=ps_g[:, :dw],
                                 func=AF.Copy, scale=SCALE)

        # ---- emb_srcT -> h1T (= W.T @ emb_src.T).
        # Only chunks 0-1 are produced in the preamble; chunks 2-7 are
        # injected into the main loop right before first use so the
        # in-order PE queue doesn't hold e-mm(0) behind 32 h1t matmuls
        # (whose last DMA dependency is the end of the src stream).
        nc.gpsimd.memset(h1t[:, N_SRC:NSP], 0.0)

        def emit_h1_chunk(j, eng):
            j0 = j * HC
            ssb = preb.tile([128, 2, HC], BF16, tag="ssb",
                            name=f"ssb{j % 2}")
            for c in range(2):
                eng.dma_start(
                    out=ssb[:, c, :],
                    in_=srcT_t[128 * c:128 * (c + 1), j0:j0 + HC])
            for half in range(2):
                ps_h = pps.tile([128, 512], F32, tag="ps_b",
                                name=f"psh{j % 2}{half}")
                for c in range(2):
                    nc.tensor.matmul(
                        ps_h[:, :500], w_sb[:, c, :],
                        ssb[:, c, half * 500:half * 500 + 500],
                        start=(c == 0), stop=(c == 1))
                if half == 0:
                    nc.scalar.copy(out=h1t[:, j0:j0 + 500],
                                   in_=ps_h[:, :500])
                else:
                    nc.vector.tensor_copy(out=h1t[:, j0 + 500:j0 + HC],
                                          in_=ps_h[:, :500])

        for j in range(2):
            emit_h1_chunk(j, nc.scalar)
        for s in range(2, 6):
            issue_bt(s)

        psR = pacc.tile([128, 512], F32)  # rows 32k: denom, 32k+1: numer

        H1_TRIG = {4: 2, 12: 3, 20: 4, 28: 5, 36: 6, 44: 7}

        def emit_reduces(u2, grp):
            # reduce-mms for a WHOLE previous group: keeps the in-order PE
            # queue from stalling the next e-matmuls behind a reduce that
            # waits on that group's exp
            for k, s in enumerate(grp):
                for ck, (o, w) in enumerate(CHK):
                    nc.tensor.matmul(psR[32 * ck:32 * ck + 2, :w],
                                     ftw[:, 2 * s:2 * s + 2],
                                     u2[:, k * NDP + o:k * NDP + o + w],
                                     start=(s == 0), stop=(s == NST - 1))

        # src tiles processed in PAIRS: the SBUF-side elementwise ops (mul,
        # max, prelu, exp) run once per pair at double width, halving their
        # fixed per-instruction costs; the stt stays per-tile (PSUM-bound).
        groups = [(2 * p, 2 * p + 1) for p in range(NST // 2)]
        groups.append((NST - 1,))
        ru_prev = None
        for grp in groups:
            n = len(grp)
            WG = NDP * n
            sm2 = pt.tile([128, WG], FP16, tag="sm", name=f"sm{grp[0] % 6}")
            for k, s in enumerate(grp):
                btT = bt_tiles.pop(s)
                if s in H1_TRIG:
                    emit_h1_chunk(H1_TRIG[s], nc.gpsimd)
                if s + 6 < NST:
                    issue_bt(s + 6)

                psE = mps.tile([128, 1536], F32, tag="psE")
                for (o, w) in CHK:
                    nc.tensor.matmul(psE[:, o:o + w],
                                     h1t[:, 128 * s:128 * (s + 1)],
                                     gts[:, o:o + w], start=True, stop=True)

                # mask fused into the single PSUM-consuming pass:
                # kept (b>0): BIG*b = +huge -> min picks psE
                # masked (b<0): BIG*b = -huge -> -inf fp16 -> exp() == 0
                nc.vector.scalar_tensor_tensor(
                    out=sm2[:, k * NDP:(k + 1) * NDP], in0=btT, scalar=BIG,
                    in1=psE[:, :NDP], op0=OP.mult, op1=OP.min)

            # pair-wide lrelu split: DVE (mul 4x + max 2x) on cols [:D_DVE]
            # of each tile, ACT Prelu (same table set as exp -> no
            # ACT_TABLE_LOAD swaps) on the rest
            smv = sm2[:, :].rearrange("p (n c) -> p n c", n=n)
            y2 = pt.tile([128, n * D_DVE], FP16, tag="y",
                         name=f"y{grp[0] % 6}")
            y2v = y2[:, :].rearrange("p (n c) -> p n c", n=n)
            t2 = pt.tile([128, WG], FP16, tag="t", name=f"t{grp[0] % 6}")
            t2v = t2[:, :].rearrange("p (n c) -> p n c", n=n)
            nc.vector.tensor_scalar_mul(y2v, smv[:, :, :D_DVE], 0.2)
            nc.vector.tensor_max(t2v[:, :, :D_DVE], smv[:, :, :D_DVE], y2v)
            nc.scalar.activation(out=t2v[:, :, D_DVE:],
                                 in_=smv[:, :, D_DVE:],
                                 func=AF.Prelu, alpha=0.2)
            u2 = pu.tile([128, WG], BF16, tag="u", name=f"u{grp[0] % 6}")
            nc.scalar.activation(out=u2, in_=t2, func=AF.Exp)

            if ru_prev is not None:
                emit_reduces(*ru_prev)
            ru_prev = (u2, grp)

        emit_reduces(*ru_prev)

        # ---- finals: out = numer / denom on 3 partitions; denom rows 32k /
        # numer rows 32k+1 gathered straight from PSUM by two strided DMAs
        # (DMA descriptors handle partition strides; DVE ops cannot)
        rsb = psm.tile([66, 512], F32, tag="rsb")
        nc.scalar.copy(out=rsb, in_=psR[:66, :])
        d3 = psm.tile([3, 512], F32, tag="d3")
        n3 = psm.tile([3, 512], F32, tag="n3")
        nc.sync.dma_start(out=d3, in_=rsb[0:65:32, :])
        nc.sync.dma_start(out=n3, in_=rsb[1:66:32, :])
        rec3 = psm.tile([3, 512], F32, tag="rec3")
        scr3 = psm.tile([3, 512], F32, tag="scr3")
        nc.vector.reciprocal_approx_accurate(out=rec3, in_=d3, scratch=scr3)
        o3 = psm.tile([3, 512], F32, tag="o3")
        nc.vector.tensor_mul(o3, n3, rec3)
        for k, (o, w) in enumerate(CHK):
            we = min(o + w, ND) - o
            nc.sync.dma_start(out=out_t[:, o:o + we], in_=o3[k:k + 1, :we])

    nc.compile()
    return nc


def _get_nc():
    if "nc" not in _CACHE:
        _CACHE["nc"] = _build_nc()
    return _CACHE["nc"]


def kernel(bias, emb_dest, emb_src, feature_src, W, W2, _trace=False):
    import ml_dtypes
    from concourse.bass_utils import run_bass_kernel_spmd

    BF = ml_dtypes.bfloat16
    F8 = ml_dtypes.float8_e5m2

    bias = np.ascontiguousarray(bias, dtype=np.float32)
    emb_dest = np.ascontiguousarray(emb_dest, dtype=np.float32)
    emb_src = np.ascontiguousarray(emb_src, dtype=np.float32)
    ft = np.ascontiguousarray(feature_src, dtype=np.float32).reshape(-1)
    W = np.ascontiguousarray(W, dtype=np.float32)
    W2 = np.ascontiguousarray(W2, dtype=np.float32)

    nan_ind = np.isnan(ft)
    if nan_ind.any():
        # NaN source features: zero the feature and mask out the column
        # (matches reference semantics). Never hit for randn inputs.
        ft = np.where(nan_ind, 0.0, ft)
        bias = np.where(nan_ind.reshape(1, -1), -1.0, bias)

    srcT = np.ascontiguousarray(emb_src.T.astype(BF))    # [256, 8000] bf16
    ftp = np.zeros(NSP, dtype=np.float32)
    ftp[:N_SRC] = ft
    ft_cols = np.ascontiguousarray(ftp.reshape(NST, 128).T)  # [128, 63]

    # bias staged as fp8e5 (only the sign is consumed on device; e5m2 is
    # sign-exact for |x| >= 2^-17, i.e. all but ~1e-5 of randn mass)
    bias8 = bias.astype(F8)

    Wb = W.astype(BF)
    nc = _get_nc()
    in_maps = []
    for i in range(N_CORES):
        r0 = i * ND
        slabT = np.zeros((NSP, NDP), dtype=F8)
        slabT[:N_SRC, :ND] = bias8[r0:r0 + ND].T
        in_maps.append({
            "biasT": slabT.reshape(NST, 128, NDP),
            "emb_destT": np.ascontiguousarray(emb_dest[r0:r0 + ND].T.astype(BF)),
            "emb_srcT": srcT,
            "ft_cols": ft_cols,
            "W": Wb,
            "WT": np.ascontiguousarray(Wb.T),
            "W2": W2.astype(BF),
        })
    res = run_bass_kernel_spmd(nc, in_maps, list(range(N_CORES)),
                               trace=_trace)
    out = np.concatenate(
        [res.results[i]["out"].reshape(ND, 1) for i in range(N_CORES)], axis=0)
    if _trace:
        return out, res
    return out


# revision 16
# speedup vs baseline: 1.1611x; 1.1611x over previous
"""Trainium2 Bass kernel for nn_AttentionLayer (GAT-style masked attention).

Computes, for full inputs:
    h1 = emb_src @ W                      [8000, 128]
    g  = emb_dest @ (W @ W2)              [10000, 128]
    e  = g @ h1.T                         [10000, 8000]
    s  = lrelu(e, 0.2) * (1/sqrt(128))    masked to -inf where bias <= 0
    att = softmax(s, axis=1)
    out = att @ ft                        [10000, 1]   (ft = nan-cleaned feature_src)

Sharding: N_dest split across 8 NeuronCores (1250 rows each); emb_src /
feature_src / W / W2 replicated. No collectives. Softmax is unnormalized
(numer/denom) — no max subtraction needed since |scale*lrelu(e)| <= ~15.

Layout: TRANSPOSED on-device — scores are computed as e.T tiles
[src=partition, dest=free] so that BOTH softmax reductions (denominator
sum(u) and numerator sum(u*ft)) run on the Tensor engine as accumulating
matmuls with lhsT = [ones | ft_chunk].

v2 elementwise chain (vs v1): the bias mask is fused into the single
PSUM-consuming DVE pass — sm = min(BIG*biasT, psE) via scalar_tensor_tensor
(masked entries -> -inf fp16, kept -> psE). This deletes v1's separate ACT
mask-gen Relu pass entirely. LeakyReLU is then y = 0.2*sm (on GPSIMD, where
one-input tensor_scalar is cheap) and t = max(sm, y) column-split between
DVE (2x mode) and GPSIMD, tuned so DVE ~= GPSIMD ~= ACT(exp) per tile.

Dtype staging (host is layout/dtype only): bias is staged transposed
tile-contiguous [63,128,1280] as float8_e5m2 (sign-exact down to 2^-17;
only the sign is consumed on device) — 4x less HBM than f32. Embeddings
and weights are staged bf16 (the matmuls already ran at bf16 precision
in v1; this halves the 8MB src stream and runs preamble matmuls at
1 cyc/row instead of fp32r's 4).

Per-core device pipeline, per src tile s (63 tiles):
    DMA:   btT    = bias.T tile (fp8e5)      [128,1250]
    PE:    psE    = h1T_s.T @ gts            f32 PSUM (3 bank chunks)
    DVE:   sm     = (BIG * btT) min psE      fp16 (stt, 1x: the only PSUM pass)
    GPS:   y      = 0.2 * sm                 fp16
    DVE:   t[:, :D]  = max(sm, y)            (2x mode)
    GPS:   t[:, D:]  = max(sm, y)
    ACT:   u      = Exp(t) -> bf16           (exp(-inf) == 0)
    PE:    psR   += [ones | ft_s].T @ u      (emitted one tile LATE so the
                                             in-order PE queue never stalls
                                             the next e-matmul behind it)
Final: out = psR[num] / psR[den] per dest column, three row DMAs.
"""
import os
import sys

sys.path.insert(0, "/opt/trn_rl_repo")

import numpy as np

_CACHE = {}

N_DEST, N_SRC, IN_DIM, HID = 10000, 8000, 256, 128
N_CORES = 8
ND = N_DEST // N_CORES            # 1250 dest rows per core
NDP = 1250                        # dest width (free axis of transposed tiles)
NSP = 8064                        # src padded to 63 full 128-row tiles
NST = NSP // 128                  # 63 src tiles
SCALE = float(1.0 / np.sqrt(np.float32(HID)))

HC = 1000                         # h1T build chunk width
N_HC = N_SRC // HC                # 8

CHK = [(0, 512), (512, 512), (1024, NDP - 1024)]   # dest chunks (PSUM banks)
BIG = 1e30                        # mask scale: min(BIG*b, e) = e keep / -inf mask

D_DVE = 590                       # lrelu columns on DVE (mul+max); rest on ACT Prelu


def _build_nc():
    import concourse.bass as bass
    import concourse.tile as tile
    from concourse import bacc, mybir
    from contextlib import ExitStack

    F32 = mybir.dt.float32
    BF16 = mybir.dt.bfloat16
    FP16 = mybir.dt.float16
    F8E5 = mybir.dt.float8e5
    AF = mybir.ActivationFunctionType
    OP = mybir.AluOpType

    nc = bacc.Bacc("TRN2", target_bir_lowering=False, debug=False,
                   num_devices=N_CORES)

    bias_t = nc.declare_dram_parameter("biasT", [NST, 128, NDP], F8E5,
                                       isOutput=False)
    destT_t = nc.declare_dram_parameter("emb_destT", [IN_DIM, ND], BF16,
                                        isOutput=False)
    srcT_t = nc.declare_dram_parameter("emb_srcT", [IN_DIM, N_SRC], BF16,
                                       isOutput=False)
    ftc_t = nc.declare_dram_parameter("ft_cols", [128, NST], F32,
                                      isOutput=False)
    w_t = nc.declare_dram_parameter("W", [IN_DIM, HID], BF16, isOutput=False)
    wt_t = nc.declare_dram_parameter("WT", [HID, IN_DIM], BF16, isOutput=False)
    w2_t = nc.declare_dram_parameter("W2", [HID, HID], BF16, isOutput=False)
    out_t = nc.declare_dram_parameter("out", [1, ND], F32, isOutput=True)

    with tile.TileContext(nc) as tc, ExitStack() as ctx:
        persist = ctx.enter_context(tc.tile_pool(name="persist", bufs=1))

        # persistent tiles
        gts = persist.tile([128, NDP], BF16)      # SCALE * g.T  [hid, dest]
        h1t = persist.tile([128, NSP], BF16)      # h1.T         [hid, src]
        ftw = persist.tile([128, 2 * NST], BF16)  # per-src-tile [ones | ft]

        # ================= main loop (pools concurrent with preamble:
        # no close barrier; main tiles start as soon as deps are ready)
        pbias = ctx.enter_context(tc.tile_pool(name="mn_bias", bufs=6))
        pt = ctx.enter_context(tc.tile_pool(name="mn_t", bufs=3))
        pu = ctx.enter_context(tc.tile_pool(name="mn_u", bufs=3))
        psm = ctx.enter_context(tc.tile_pool(name="mn_small", bufs=1))
        pacc = ctx.enter_context(
            tc.tile_pool(name="mn_acc", bufs=1, space="PSUM"))
        mps = ctx.enter_context(
            tc.tile_pool(name="mn_ps", bufs=2, space="PSUM"))
        pre = ctx.enter_context(tc.tile_pool(name="pre_sb", bufs=2))
        preb = ctx.enter_context(tc.tile_pool(name="pre_big", bufs=2))
        pps = ctx.enter_context(
            tc.tile_pool(name="pre_ps", bufs=1, space="PSUM"))

        # ---- W chunks ([K=in_sub, M=hid]), bf16 (critical path: wc -> gts)
        w_sb = pre.tile([128, 2, HID], BF16, tag="w_sb")
        for c in range(2):
            nc.sync.dma_start(out=w_sb[:, c, :],
                              in_=w_t[128 * c:128 * (c + 1), :])
        w2_sb = pre.tile([128, HID], BF16, tag="w2_sb")
        nc.sync.dma_start(out=w2_sb, in_=w2_t[:, :])
        wt_sb = pre.tile([128, IN_DIM], BF16, tag="wt_sb")
        nc.sync.dma_start(out=wt_sb, in_=wt_t[:, :])

        # ---- ftw: per src tile s, columns [2s, 2s+1] = [ones, ft_s]
        ftc_sb = pre.tile([128, NST], F32, tag="ftc_sb")
        nc.sync.dma_start(out=ftc_sb, in_=ftc_t[:, :])
        ftw_v = ftw[:, :].rearrange("p (s two) -> p s two", two=2)
        nc.gpsimd.memset(ftw_v[:, :, 0], 1.0)
        nc.vector.tensor_copy(out=ftw_v[:, :, 1], in_=ftc_sb)

        # ---- Wc = W @ W2 as [K=in_sub, M=hid] chunks (lhsT = staged W.T)
        wc_sb = pre.tile([128, 2, HID], BF16, tag="wc_sb")
        for c in range(2):
            ps_mm = pps.tile([128, 512], F32, tag="ps_b")
            nc.tensor.matmul(ps_mm[:, :HID],
                             wt_sb[:, 128 * c:128 * (c + 1)], w2_sb,
                             start=True, stop=True)
            nc.scalar.copy(out=wc_sb[:, c, :], in_=ps_mm[:, :HID])

        # ---- bias tile DMA issue (sync queue, 6-buffer ring)
        bt_tiles = {}

        def issue_bt(s):
            btT = pbias.tile([128, NDP], F8E5, tag="btT", name=f"bt{s % 6}")
            nc.sync.dma_start(out=btT, in_=bias_t[s])
            bt_tiles[s] = btT

        issue_bt(0)
        issue_bt(1)

        # ---- emb_destT -> gts (= SCALE * Wc.T @ emb_dest.T)
        # dest/src preamble streams ride the ACT DMA queue so they don't
        # serialize behind the bias tiles on the sync queue
        dsb = preb.tile([128, 2, ND], BF16, tag="dsb")
        for c in range(2):
            nc.scalar.dma_start(out=dsb[:, c, :],
                                in_=destT_t[128 * c:128 * (c + 1), :])
        for d0 in range(0, NDP, 512):
            dw = min(512, NDP - d0)
            ps_g = pps.tile([128, 512], F32, tag="ps_b")
            for c in range(2):
                nc.tensor.matmul(ps_g[:, :dw],
                                 wc_sb[:, c, :],
                                 dsb[:, c, d0:d0 + dw],
                                 start=(c == 0), stop=(c == 1))
            nc.scalar.activation(out=gts[:, d0:d0 + dw], in_=ps_g[:, :dw],
                                 func=AF.Copy, scale=SCALE)

        # ---- emb_srcT -> h1T (= W.T @ emb_src.T).
        # Only chunks 0-1 are produced in the preamble; chunks 2-7 are
        # injected into the main loop right before first use so the
        # in-order PE queue doesn't hold e-mm(0) behind 32 h1t matmuls
        # (whose last DMA dependency is the end of the src stream).
        nc.gpsimd.memset(h1t[:, N_SRC:NSP], 0.0)

        def emit_h1_chunk(j, eng):
            j0 = j * HC
            ssb = preb.tile([128, 2, HC], BF16, tag="ssb",
                            name=f"ssb{j % 2}")
            for c in range(2):
                eng.dma_start(
                    out=ssb[:, c, :],
                    in_=srcT_t[128 * c:128 * (c + 1), j0:j0 + HC])
            for half in range(2):
                ps_h = pps.tile([128, 512], F32, tag="ps_b",
                                name=f"psh{j % 2}{half}")
                for c in range(2):
                    nc.tensor.matmul(
                        ps_h[:, :500], w_sb[:, c, :],
                        ssb[:, c, half * 500:half * 500 + 500],
                        start=(c == 0), stop=(c == 1))
                if half == 0:
                    nc.scalar.copy(out=h1t[:, j0:j0 + 500],
                                   in_=ps_h[:, :500])
                else:
                    nc.vector.tensor_copy(out=h1t[:, j0 + 500:j0 + HC],
                                          in_=ps_h[:, :500])

        for j in range(2):
            emit_h1_chunk(j, nc.scalar)
        for s in range(2, 6):
            issue_bt(s)

        psR = pacc.tile([128, 512], F32)  # rows 32k: denom, 32k+1: numer

        H1_TRIG = {4: 2, 12: 3, 20: 4, 28: 5, 36: 6, 44: 7}
        for s in range(NST):
            btT = bt_tiles.pop(s)
            if s in H1_TRIG:
                emit_h1_chunk(H1_TRIG[s], nc.gpsimd)
            if s + 6 < NST:
                issue_bt(s + 6)

            psE = mps.tile([128, 1536], F32, tag="psE")
            for (o, w) in CHK:
                nc.tensor.matmul(psE[:, o:o + w],
                                 h1t[:, 128 * s:128 * (s + 1)],
                                 gts[:, o:o + w], start=True, stop=True)

            # mask fused into the single PSUM-consuming pass:
            # kept (b>0): BIG*b = +huge -> min picks psE
            # masked (b<0): BIG*b = -huge -> -inf in fp16 -> exp() == 0
            sm = pt.tile([128, NDP], FP16, tag="sm")
            nc.vector.scalar_tensor_tensor(
                out=sm, in0=btT, scalar=BIG, in1=psE[:, :NDP],
                op0=OP.mult, op1=OP.min)
            # lrelu split: DVE (mul 4x + max 2x) on cols [:D_DVE], ACT Prelu
            # (parametric_relu lives in the same table set as exp -> no
            # ACT_TABLE_LOAD swaps) on cols [D_DVE:]
            y = pt.tile([128, D_DVE], FP16, tag="y")
            nc.vector.tensor_scalar_mul(y, sm[:, :D_DVE], 0.2)
            t = pt.tile([128, NDP], FP16, tag="t")
            nc.vector.tensor_max(t[:, :D_DVE], sm[:, :D_DVE], y)
            nc.scalar.activation(out=t[:, D_DVE:], in_=sm[:, D_DVE:],
                                 func=AF.Prelu, alpha=0.2)
            u = pu.tile([128, NDP], BF16, tag="u")
            nc.scalar.activation(out=u, in_=t, func=AF.Exp)

            # reduce-mm for the PREVIOUS tile: keeps the in-order PE queue
            # from stalling e-mm(s+1) behind a reduce that waits on exp(s)
            if s > 0:
                up, sp = u_prev
                for k, (o, w) in enumerate(CHK):
                    nc.tensor.matmul(psR[32 * k:32 * k + 2, :w],
                                     ftw[:, 2 * sp:2 * sp + 2],
                                     up[:, o:o + w],
                                     start=(sp == 0), stop=False)
            u_prev = (u, s)

        up, sp = u_prev
        for k, (o, w) in enumerate(CHK):
            nc.tensor.matmul(psR[32 * k:32 * k + 2, :w],
                             ftw[:, 2 * sp:2 * sp + 2], up[:, o:o + w],
                             start=False, stop=True)

        # ---- finals: out = numer / denom on 3 partitions; denom rows 32k /
        # numer rows 32k+1 gathered straight from PSUM by two strided DMAs
        # (DMA descriptors handle partition strides; DVE ops cannot)
        rsb = psm.tile([66, 512], F32, tag="rsb")
        nc.scalar.copy(out=rsb, in_=psR[:66, :])
        d3 = psm.tile([3, 512], F32, tag="d3")
        n3 = psm.tile([3, 512], F32, tag="n3")
        nc.sync.dma_start(out=d3, in_=rsb[0:65:32, :])
        nc.sync.dma_start(out=n3, in_=rsb[1:66:32, :])
        rec3 = psm.tile([3, 512], F32, tag="rec3")
        scr3 = psm.tile([3, 512], F32, tag="scr3")
        nc.vector.reciprocal_approx_accurate(out=rec3, in_=d3, scratch=scr3)
        o3 = psm.tile([3, 512], F32, tag="o3")
        nc.vector.tensor_mul(o3, n3, rec3)
        for k, (o, w) in enumerate(CHK):
            we = min(o + w, ND) - o
            nc.sync.dma_start(out=out_t[:, o:o + we], in_=o3[k:k + 1, :we])

    nc.compile()
    return nc


def _get_nc():
    if "nc" not in _CACHE:
        _CACHE["nc"] = _build_nc()
    return _CACHE["nc"]


def kernel(bias, emb_dest, emb_src, feature_src, W, W2, _trace=False):
    import ml_dtypes
    from concourse.bass_utils import run_bass_kernel_spmd

    BF = ml_dtypes.bfloat16
    F8 = ml_dtypes.float8_e5m2

    bias = np.ascontiguousarray(bias, dtype=np.float32)
    emb_dest = np.ascontiguousarray(emb_dest, dtype=np.float32)
    emb_src = np.ascontiguousarray(emb_src, dtype=np.float32)
    ft = np.ascontiguousarray(feature_src, dtype=np.float32).reshape(-1)
    W = np.ascontiguousarray(W, dtype=np.float32)
    W2 = np.ascontiguousarray(W2, dtype=np.float32)

    nan_ind = np.isnan(ft)
    if nan_ind.any():
        # NaN source features: zero the feature and mask out the column
        # (matches reference semantics). Never hit for randn inputs.
        ft = np.where(nan_ind, 0.0, ft)
        bias = np.where(nan_ind.reshape(1, -1), -1.0, bias)

    srcT = np.ascontiguousarray(emb_src.T.astype(BF))    # [256, 8000] bf16
    ftp = np.zeros(NSP, dtype=np.float32)
    ftp[:N_SRC] = ft
    ft_cols = np.ascontiguousarray(ftp.reshape(NST, 128).T)  # [128, 63]

    # bias staged as fp8e5 (only the sign is consumed on device; e5m2 is
    # sign-exact for |x| >= 2^-17, i.e. all but ~1e-5 of randn mass)
    bias8 = bias.astype(F8)

    Wb = W.astype(BF)
    nc = _get_nc()
    in_maps = []
    for i in range(N_CORES):
        r0 = i * ND
        slabT = np.zeros((NSP, NDP), dtype=F8)
        slabT[:N_SRC, :ND] = bias8[r0:r0 + ND].T
        in_maps.append({
            "biasT": slabT.reshape(NST, 128, NDP),
            "emb_destT": np.ascontiguousarray(emb_dest[r0:r0 + ND].T.astype(BF)),
            "emb_srcT": srcT,
            "ft_cols": ft_cols,
            "W": Wb,
            "WT": np.ascontiguousarray(Wb.T),
            "W2": W2.astype(BF),
        })
    res = run_bass_kernel_spmd(nc, in_maps, list(range(N_CORES)),
                               trace=_trace)
    out = np.concatenate(
        [res.results[i]["out"].reshape(ND, 1) for i in range(N_CORES)], axis=0)
    if _trace:
        return out, res
    return out


# revision 18
# speedup vs baseline: 1.1888x; 1.0239x over previous
"""Trainium2 Bass kernel for nn_AttentionLayer (GAT-style masked attention).

Computes, for full inputs:
    h1 = emb_src @ W                      [8000, 128]
    g  = emb_dest @ (W @ W2)              [10000, 128]
    e  = g @ h1.T                         [10000, 8000]
    s  = lrelu(e, 0.2) * (1/sqrt(128))    masked to -inf where bias <= 0
    att = softmax(s, axis=1)
    out = att @ ft                        [10000, 1]   (ft = nan-cleaned feature_src)

Sharding: N_dest split across 8 NeuronCores (1250 rows each); emb_src /
feature_src / W / W2 replicated. No collectives. Softmax is unnormalized
(numer/denom) — no max subtraction needed since |scale*lrelu(e)| <= ~15.

Layout: TRANSPOSED on-device — scores are computed as e.T tiles
[src=partition, dest=free] so that BOTH softmax reductions (denominator
sum(u) and numerator sum(u*ft)) run on the Tensor engine as accumulating
matmuls with lhsT = [ones | ft_chunk].

v2 elementwise chain (vs v1): the bias mask is fused into the single
PSUM-consuming DVE pass — sm = min(BIG*biasT, psE) via scalar_tensor_tensor
(masked entries -> -inf fp16, kept -> psE). This deletes v1's separate ACT
mask-gen Relu pass entirely. LeakyReLU is then y = 0.2*sm (on GPSIMD, where
one-input tensor_scalar is cheap) and t = max(sm, y) column-split between
DVE (2x mode) and GPSIMD, tuned so DVE ~= GPSIMD ~= ACT(exp) per tile.

Dtype staging (host is layout/dtype only): bias is staged transposed
tile-contiguous [63,128,1280] as float8_e5m2 (sign-exact down to 2^-17;
only the sign is consumed on device) — 4x less HBM than f32. Embeddings
and weights are staged bf16 (the matmuls already ran at bf16 precision
in v1; this halves the 8MB src stream and runs preamble matmuls at
1 cyc/row instead of fp32r's 4).

Per-core device pipeline, per src tile s (63 tiles):
    DMA:   btT    = bias.T tile (fp8e5)      [128,1250]
    PE:    psE    = h1T_s.T @ gts            f32 PSUM (3 bank chunks)
    DVE:   sm     = (BIG * btT) min psE      fp16 (stt, 1x: the only PSUM pass)
    GPS:   y      = 0.2 * sm                 fp16
    DVE:   t[:, :D]  = max(sm, y)            (2x mode)
    GPS:   t[:, D:]  = max(sm, y)
    ACT:   u      = Exp(t) -> bf16           (exp(-inf) == 0)
    PE:    psR   += [ones | ft_s].T @ u      (emitted one tile LATE so the
                                             in-order PE queue never stalls
                                             the next e-matmul behind it)
Final: out = psR[num] / psR[den] per dest column, three row DMAs.
"""
import os
import sys

sys.path.insert(0, "/opt/trn_rl_repo")

import numpy as np

_CACHE = {}

N_DEST, N_SRC, IN_DIM, HID = 10000, 8000, 256, 128
N_CORES = 8
ND = N_DEST // N_CORES            # 1250 dest rows per core
NDP = 1250                        # dest width (free axis of transposed tiles)
NSP = 8064                        # src padded to 63 full 128-row tiles
NST = NSP // 128                  # 63 src tiles
SCALE = float(1.0 / np.sqrt(np.float32(HID)))

HC = 1000                         # h1T build chunk width
N_HC = N_SRC // HC                # 8

CHK = [(0, 512), (512, 512), (1024, NDP - 1024)]   # dest chunks (PSUM banks)
BIG = 1e30                        # mask scale: min(BIG*b, e) = e keep / -inf mask

D_DVE = 590                       # lrelu columns on DVE (mul+max); rest on ACT Prelu


def _build_nc():
    import concourse.bass as bass
    import concourse.tile as tile
    from concourse import bacc, mybir
    from contextlib import ExitStack

    F32 = mybir.dt.float32
    BF16 = mybir.dt.bfloat16
    FP16 = mybir.dt.float16
    F8E5 = mybir.dt.float8e5
    AF = mybir.ActivationFunctionType
    OP = mybir.AluOpType

    nc = bacc.Bacc("TRN2", target_bir_lowering=False, debug=False,
                   num_devices=N_CORES)

    bias_t = nc.declare_dram_parameter("biasT", [NST, 128, NDP], F8E5,
                                       isOutput=False)
    destT_t = nc.declare_dram_parameter("emb_destT", [IN_DIM, ND], BF16,
                                        isOutput=False)
    srcT_t = nc.declare_dram_parameter("emb_srcT", [IN_DIM, N_SRC], BF16,
                                       isOutput=False)
    ftc_t = nc.declare_dram_parameter("ft_cols", [128, NST], F32,
                                      isOutput=False)
    w_t = nc.declare_dram_parameter("W", [IN_DIM, HID], BF16, isOutput=False)
    wt_t = nc.declare_dram_parameter("WT", [HID, IN_DIM], BF16, isOutput=False)
    w2_t = nc.declare_dram_parameter("W2", [HID, HID], BF16, isOutput=False)
    out_t = nc.declare_dram_parameter("out", [1, ND], F32, isOutput=True)

    with tile.TileContext(nc) as tc, ExitStack() as ctx:
        persist = ctx.enter_context(tc.tile_pool(name="persist", bufs=1))

        # persistent tiles
        gts = persist.tile([128, NDP], BF16)      # SCALE * g.T  [hid, dest]
        h1t = persist.tile([128, NSP], BF16)      # h1.T         [hid, src]
        ftw = persist.tile([128, 2 * NST], BF16)  # per-src-tile [ones | ft]

        # ================= main loop (pools concurrent with preamble:
        # no close barrier; main tiles start as soon as deps are ready)
        pbias = ctx.enter_context(tc.tile_pool(name="mn_bias", bufs=6))
        pt = ctx.enter_context(tc.tile_pool(name="mn_t", bufs=3))
        pu = ctx.enter_context(tc.tile_pool(name="mn_u", bufs=3))
        psm = ctx.enter_context(tc.tile_pool(name="mn_small", bufs=1))
        pacc = ctx.enter_context(
            tc.tile_pool(name="mn_acc", bufs=1, space="PSUM"))
        mps = ctx.enter_context(
            tc.tile_pool(name="mn_ps", bufs=2, space="PSUM"))
        pre = ctx.enter_context(tc.tile_pool(name="pre_sb", bufs=2))
        preb = ctx.enter_context(tc.tile_pool(name="pre_big", bufs=2))
        pps = ctx.enter_context(
            tc.tile_pool(name="pre_ps", bufs=1, space="PSUM"))

        # ---- W chunks ([K=in_sub, M=hid]), bf16 (critical path: wc -> gts)
        w_sb = pre.tile([128, 2, HID], BF16, tag="w_sb")
        for c in range(2):
            nc.sync.dma_start(out=w_sb[:, c, :],
                              in_=w_t[128 * c:128 * (c + 1), :])
        w2_sb = pre.tile([128, HID], BF16, tag="w2_sb")
        nc.sync.dma_start(out=w2_sb, in_=w2_t[:, :])
        wt_sb = pre.tile([128, IN_DIM], BF16, tag="wt_sb")
        nc.sync.dma_start(out=wt_sb, in_=wt_t[:, :])

        # ---- ftw: per src tile s, columns [2s, 2s+1] = [ones, ft_s]
        ftc_sb = pre.tile([128, NST], F32, tag="ftc_sb")
        nc.sync.dma_start(out=ftc_sb, in_=ftc_t[:, :])
        ftw_v = ftw[:, :].rearrange("p (s two) -> p s two", two=2)
        nc.gpsimd.memset(ftw_v[:, :, 0], 1.0)
        nc.vector.tensor_copy(out=ftw_v[:, :, 1], in_=ftc_sb)

        # ---- Wc = W @ W2 as [K=in_sub, M=hid] chunks (lhsT = staged W.T)
        # wc/gts preamble matmuls borrow the double-buffered main-loop PSUM
        # pool (mps) so each chunk's copy overlaps the next chunk's matmul
        # (pps has bufs=1 and would serialize PE->ACT->PE)
        wc_sb = pre.tile([128, 2, HID], BF16, tag="wc_sb")
        for c in range(2):
            ps_mm = mps.tile([128, 1536], F32, tag="psE", name=f"wc{c}")
            nc.tensor.matmul(ps_mm[:, :HID],
                             wt_sb[:, 128 * c:128 * (c + 1)], w2_sb,
                             start=True, stop=True)
            nc.scalar.copy(out=wc_sb[:, c, :], in_=ps_mm[:, :HID])

        # ---- bias tile DMA issue (sync queue, 6-buffer ring)
        bt_tiles = {}

        def issue_bt(s):
            btT = pbias.tile([128, NDP], F8E5, tag="btT", name=f"bt{s % 6}")
            nc.sync.dma_start(out=btT, in_=bias_t[s])
            bt_tiles[s] = btT

        issue_bt(0)
        issue_bt(1)

        # ---- emb_destT -> gts (= SCALE * Wc.T @ emb_dest.T)
        # dest/src preamble streams ride the ACT DMA queue so they don't
        # serialize behind the bias tiles on the sync queue
        dsb = preb.tile([128, 2, ND], BF16, tag="dsb")
        for c in range(2):
            nc.scalar.dma_start(out=dsb[:, c, :],
                                in_=destT_t[128 * c:128 * (c + 1), :])
        for d0 in range(0, NDP, 512):
            dw = min(512, NDP - d0)
            ps_g = mps.tile([128, 1536], F32, tag="psE", name=f"g{d0}")
            for c in range(2):
                nc.tensor.matmul(ps_g[:, :dw],
                                 wc_sb[:, c, :],
                                 dsb[:, c, d0:d0 + dw],
                                 start=(c == 0), stop=(c == 1))
            nc.scalar.activation(out=gts[:, d0:d0 + dw], in_=ps_g[:, :dw],
                                 func=AF.Copy, scale=SCALE)

        # ---- emb_srcT -> h1T (= W.T @ emb_src.T).
        # Only chunks 0-1 are produced in the preamble; chunks 2-7 are
        # injected into the main loop right before first use so the
        # in-order PE queue doesn't hold e-mm(0) behind 32 h1t matmuls
        # (whose last DMA dependency is the end of the src stream).
        nc.gpsimd.memset(h1t[:, N_SRC:NSP], 0.0)

        def emit_h1_chunk(j, eng):
            j0 = j * HC
            ssb = preb.tile([128, 2, HC], BF16, tag="ssb",
                            name=f"ssb{j % 2}")
            for c in range(2):
                eng.dma_start(
                    out=ssb[:, c, :],
                    in_=srcT_t[128 * c:128 * (c + 1), j0:j0 + HC])
            for half in range(2):
                ps_h = pps.tile([128, 512], F32, tag="ps_b",
                                name=f"psh{j % 2}{half}")
                for c in range(2):
                    nc.tensor.matmul(
                        ps_h[:, :500], w_sb[:, c, :],
                        ssb[:, c, half * 500:half * 500 + 500],
                        start=(c == 0), stop=(c == 1))
                if half == 0:
                    nc.scalar.copy(out=h1t[:, j0:j0 + 500],
                                   in_=ps_h[:, :500])
                else:
                    nc.vector.tensor_copy(out=h1t[:, j0 + 500:j0 + HC],
                                          in_=ps_h[:, :500])

        for j in range(2):
            emit_h1_chunk(j, nc.scalar)
        for s in range(2, 6):
            issue_bt(s)

        psR = pacc.tile([128, 512], F32)  # rows 32k: denom, 32k+1: numer

        H1_TRIG = {4: 2, 12: 3, 20: 4, 28: 5, 36: 6, 44: 7}
        for s in range(NST):
            btT = bt_tiles.pop(s)
            if s in H1_TRIG:
                emit_h1_chunk(H1_TRIG[s], nc.gpsimd)
            if s + 6 < NST:
                issue_bt(s + 6)

            psE = mps.tile([128, 1536], F32, tag="psE")
            for (o, w) in CHK:
                nc.tensor.matmul(psE[:, o:o + w],
                                 h1t[:, 128 * s:128 * (s + 1)],
                                 gts[:, o:o + w], start=True, stop=True)

            # mask fused into the single PSUM-consuming pass:
            # kept (b>0): BIG*b = +huge -> min picks psE
            # masked (b<0): BIG*b = -huge -> -inf in fp16 -> exp() == 0
            sm = pt.tile([128, NDP], FP16, tag="sm")
            nc.vector.scalar_tensor_tensor(
                out=sm, in0=btT, scalar=BIG, in1=psE[:, :NDP],
                op0=OP.mult, op1=OP.min)
            # lrelu split: DVE (mul 4x + max 2x) on cols [:D_DVE], ACT Prelu
            # (parametric_relu lives in the same table set as exp -> no
            # ACT_TABLE_LOAD swaps) on cols [D_DVE:]
            y = pt.tile([128, D_DVE], FP16, tag="y")
            nc.vector.tensor_scalar_mul(y, sm[:, :D_DVE], 0.2)
            t = pt.tile([128, NDP], FP16, tag="t")
            nc.vector.tensor_max(t[:, :D_DVE], sm[:, :D_DVE], y)
            nc.scalar.activation(out=t[:, D_DVE:], in_=sm[:, D_DVE:],
                                 func=AF.Prelu, alpha=0.2)
            u = pu.tile([128, NDP], BF16, tag="u")
            nc.scalar.activation(out=u, in_=t, func=AF.Exp)

            # reduce-mm for the PREVIOUS tile: keeps the in-order PE queue
            # from stalling e-mm(s+1) behind a reduce that waits on exp(s)
            if s > 0:
                up, sp = u_prev
                for k, (o, w) in enumerate(CHK):
                    nc.tensor.matmul(psR[32 * k:32 * k + 2, :w],
                                     ftw[:, 2 * sp:2 * sp + 2],
                                     up[:, o:o + w],
                                     start=(sp == 0), stop=False)
            u_prev = (u, s)

        up, sp = u_prev
        for k, (o, w) in enumerate(CHK):
            nc.tensor.matmul(psR[32 * k:32 * k + 2, :w],
                             ftw[:, 2 * sp:2 * sp + 2], up[:, o:o + w],
                             start=False, stop=True)

        # ---- finals: out = numer / denom on 3 partitions; denom rows 32k /
        # numer rows 32k+1 gathered straight from PSUM by two strided DMAs
        # (DMA descriptors handle partition strides; DVE ops cannot)
        rsb = psm.tile([66, 512], F32, tag="rsb")
        nc.scalar.copy(out=rsb, in_=psR[:66, :])
        d3 = psm.tile([3, 512], F32, tag="d3")
        n3 = psm.tile([3, 512], F32, tag="n3")
        nc.sync.dma_start(out=d3, in_=rsb[0:65:32, :])
        nc.sync.dma_start(out=n3, in_=rsb[1:66:32, :])
        rec3 = psm.tile([3, 512], F32, tag="rec3")
        scr3 = psm.tile([3, 512], F32, tag="scr3")
        nc.vector.reciprocal_approx_accurate(out=rec3, in_=d3, scratch=scr3)
        o3 = psm.tile([3, 512], F32, tag="o3")
        nc.vector.tensor_mul(o3, n3, rec3)
        for k, (o, w) in enumerate(CHK):
            we = min(o + w, ND) - o
            nc.sync.dma_start(out=out_t[:, o:o + we], in_=o3[k:k + 1, :we])

    nc.compile()
    return nc


def _get_nc():
    if "nc" not in _CACHE:
        _CACHE["nc"] = _build_nc()
    return _CACHE["nc"]


def kernel(bias, emb_dest, emb_src, feature_src, W, W2, _trace=False):
    import ml_dtypes
    from concourse.bass_utils import run_bass_kernel_spmd

    BF = ml_dtypes.bfloat16
    F8 = ml_dtypes.float8_e5m2

    bias = np.ascontiguousarray(bias, dtype=np.float32)
    emb_dest = np.ascontiguousarray(emb_dest, dtype=np.float32)
    emb_src = np.ascontiguousarray(emb_src, dtype=np.float32)
    ft = np.ascontiguousarray(feature_src, dtype=np.float32).reshape(-1)
    W = np.ascontiguousarray(W, dtype=np.float32)
    W2 = np.ascontiguousarray(W2, dtype=np.float32)

    nan_ind = np.isnan(ft)
    if nan_ind.any():
        # NaN source features: zero the feature and mask out the column
        # (matches reference semantics). Never hit for randn inputs.
        ft = np.where(nan_ind, 0.0, ft)
        bias = np.where(nan_ind.reshape(1, -1), -1.0, bias)

    srcT = np.ascontiguousarray(emb_src.T.astype(BF))    # [256, 8000] bf16
    ftp = np.zeros(NSP, dtype=np.float32)
    ftp[:N_SRC] = ft
    ft_cols = np.ascontiguousarray(ftp.reshape(NST, 128).T)  # [128, 63]

    # bias staged as fp8e5 (only the sign is consumed on device; e5m2 is
    # sign-exact for |x| >= 2^-17, i.e. all but ~1e-5 of randn mass)
    bias8 = bias.astype(F8)

    Wb = W.astype(BF)
    nc = _get_nc()
    in_maps = []
    for i in range(N_CORES):
        r0 = i * ND
        slabT = np.zeros((NSP, NDP), dtype=F8)
        slabT[:N_SRC, :ND] = bias8[r0:r0 + ND].T
        in_maps.append({
            "biasT": slabT.reshape(NST, 128, NDP),
            "emb_destT": np.ascontiguousarray(emb_dest[r0:r0 + ND].T.astype(BF)),
            "emb_srcT": srcT,
            "ft_cols": ft_cols,
            "W": Wb,
            "WT": np.ascontiguousarray(Wb.T),
            "W2": W2.astype(BF),
        })
    res = run_bass_kernel_spmd(nc, in_maps, list(range(N_CORES)),
                               trace=_trace)
    out = np.concatenate(
        [res.results[i]["out"].reshape(ND, 1) for i in range(N_CORES)], axis=0)
    if _trace:
        return out, res
    return out


# revision 20
# speedup vs baseline: 1.1983x; 1.0080x over previous
"""Trainium2 Bass kernel for nn_AttentionLayer (GAT-style masked attention).

Computes, for full inputs:
    h1 = emb_src @ W                      [8000, 128]
    g  = emb_dest @ (W @ W2)              [10000, 128]
    e  = g @ h1.T                         [10000, 8000]
    s  = lrelu(e, 0.2) * (1/sqrt(128))    masked to -inf where bias <= 0
    att = softmax(s, axis=1)
    out = att @ ft                        [10000, 1]   (ft = nan-cleaned feature_src)

Sharding: N_dest split across 8 NeuronCores (1250 rows each); emb_src /
feature_src / W / W2 replicated. No collectives. Softmax is unnormalized
(numer/denom) — no max subtraction needed since |scale*lrelu(e)| <= ~15.

Layout: TRANSPOSED on-device — scores are computed as e.T tiles
[src=partition, dest=free] so that BOTH softmax reductions (denominator
sum(u) and numerator sum(u*ft)) run on the Tensor engine as accumulating
matmuls with lhsT = [ones | ft_chunk].

v2 elementwise chain (vs v1): the bias mask is fused into the single
PSUM-consuming DVE pass — sm = min(BIG*biasT, psE) via scalar_tensor_tensor
(masked entries -> -inf fp16, kept -> psE). This deletes v1's separate ACT
mask-gen Relu pass entirely. LeakyReLU is then y = 0.2*sm (on GPSIMD, where
one-input tensor_scalar is cheap) and t = max(sm, y) column-split between
DVE (2x mode) and GPSIMD, tuned so DVE ~= GPSIMD ~= ACT(exp) per tile.

Dtype staging (host is layout/dtype only): bias is staged transposed
tile-contiguous [63,128,1280] as float8_e5m2 (sign-exact down to 2^-17;
only the sign is consumed on device) — 4x less HBM than f32. Embeddings
and weights are staged bf16 (the matmuls already ran at bf16 precision
in v1; this halves the 8MB src stream and runs preamble matmuls at
1 cyc/row instead of fp32r's 4).

Per-core device pipeline, per src tile s (63 tiles):
    DMA:   btT    = bias.T tile (fp8e5)      [128,1250]
    PE:    psE    = h1T_s.T @ gts            f32 PSUM (3 bank chunks)
    DVE:   sm     = (BIG * btT) min psE      fp16 (stt, 1x: the only PSUM pass)
    GPS:   y      = 0.2 * sm                 fp16
    DVE:   t[:, :D]  = max(sm, y)            (2x mode)
    GPS:   t[:, D:]  = max(sm, y)
    ACT:   u      = Exp(t) -> bf16           (exp(-inf) == 0)
    PE:    psR   += [ones | ft_s].T @ u      (emitted one tile LATE so the
                                             in-order PE queue never stalls
                                             the next e-matmul behind it)
Final: out = psR[num] / psR[den] per dest column, three row DMAs.
"""
import os
import sys

sys.path.insert(0, "/opt/trn_rl_repo")

import numpy as np

_CACHE = {}

N_DEST, N_SRC, IN_DIM, HID = 10000, 8000, 256, 128
N_CORES = 8
ND = N_DEST // N_CORES            # 1250 dest rows per core
NDP = 1250                        # dest width (free axis of transposed tiles)
NSP = 8064                        # src padded to 63 full 128-row tiles
NST = NSP // 128                  # 63 src tiles
SCALE = float(1.0 / np.sqrt(np.float32(HID)))

HC = 1000                         # h1T build chunk width
N_HC = N_SRC // HC                # 8

CHK = [(0, 512), (512, 512), (1024, NDP - 1024)]   # dest chunks (PSUM banks)
BIG = 1e30                        # mask scale: min(BIG*b, e) = e keep / -inf mask

D_DVE = 590                       # lrelu columns on DVE (mul+max); rest on ACT Prelu


def _build_nc():
    import concourse.bass as bass
    import concourse.tile as tile
    from concourse import bacc, mybir
    from contextlib import ExitStack

    F32 = mybir.dt.float32
    BF16 = mybir.dt.bfloat16
    FP16 = mybir.dt.float16
    F8E5 = mybir.dt.float8e5
    AF = mybir.ActivationFunctionType
    OP = mybir.AluOpType

    nc = bacc.Bacc("TRN2", target_bir_lowering=False, debug=False,
                   num_devices=N_CORES)

    bias_t = nc.declare_dram_parameter("biasT", [NST, 128, NDP], F8E5,
                                       isOutput=False)
    destT_t = nc.declare_dram_parameter("emb_destT", [IN_DIM, ND], BF16,
                                        isOutput=False)
    srcT_t = nc.declare_dram_parameter("emb_srcT", [IN_DIM, N_SRC], BF16,
                                       isOutput=False)
    ftc_t = nc.declare_dram_parameter("ft_cols", [128, NST], F32,
                                      isOutput=False)
    w_t = nc.declare_dram_parameter("W", [IN_DIM, HID], BF16, isOutput=False)
    wt_t = nc.declare_dram_parameter("WT", [HID, IN_DIM], BF16, isOutput=False)
    w2_t = nc.declare_dram_parameter("W2", [HID, HID], BF16, isOutput=False)
    out_t = nc.declare_dram_parameter("out", [1, ND], F32, isOutput=True)

    with tile.TileContext(nc) as tc, ExitStack() as ctx:
        persist = ctx.enter_context(tc.tile_pool(name="persist", bufs=1))

        # persistent tiles
        gts = persist.tile([128, NDP], BF16)      # SCALE * g.T  [hid, dest]
        h1t = persist.tile([128, NSP], BF16)      # h1.T         [hid, src]
        ftw = persist.tile([128, 2 * NST], BF16)  # per-src-tile [ones | ft]

        # ================= main loop (pools concurrent with preamble:
        # no close barrier; main tiles start as soon as deps are ready)
        pbias = ctx.enter_context(tc.tile_pool(name="mn_bias", bufs=8))
        pt = ctx.enter_context(tc.tile_pool(name="mn_t", bufs=3))
        pu = ctx.enter_context(tc.tile_pool(name="mn_u", bufs=3))
        psm = ctx.enter_context(tc.tile_pool(name="mn_small", bufs=1))
        pacc = ctx.enter_context(
            tc.tile_pool(name="mn_acc", bufs=1, space="PSUM"))
        mps = ctx.enter_context(
            tc.tile_pool(name="mn_ps", bufs=2, space="PSUM"))
        pre = ctx.enter_context(tc.tile_pool(name="pre_sb", bufs=2))
        preb = ctx.enter_context(tc.tile_pool(name="pre_big", bufs=2))
        pps = ctx.enter_context(
            tc.tile_pool(name="pre_ps", bufs=1, space="PSUM"))

        # ---- W chunks ([K=in_sub, M=hid]), bf16 (critical path: wc -> gts)
        w_sb = pre.tile([128, 2, HID], BF16, tag="w_sb")
        for c in range(2):
            nc.sync.dma_start(out=w_sb[:, c, :],
                              in_=w_t[128 * c:128 * (c + 1), :])
        w2_sb = pre.tile([128, HID], BF16, tag="w2_sb")
        nc.sync.dma_start(out=w2_sb, in_=w2_t[:, :])
        wt_sb = pre.tile([128, IN_DIM], BF16, tag="wt_sb")
        nc.sync.dma_start(out=wt_sb, in_=wt_t[:, :])

        # ---- ftw: per src tile s, columns [2s, 2s+1] = [ones, ft_s]
        ftc_sb = pre.tile([128, NST], F32, tag="ftc_sb")
        nc.sync.dma_start(out=ftc_sb, in_=ftc_t[:, :])
        ftw_v = ftw[:, :].rearrange("p (s two) -> p s two", two=2)
        nc.gpsimd.memset(ftw_v[:, :, 0], 1.0)
        nc.vector.tensor_copy(out=ftw_v[:, :, 1], in_=ftc_sb)

        # ---- Wc = W @ W2 as [K=in_sub, M=hid] chunks (lhsT = staged W.T)
        # wc/gts preamble matmuls borrow the double-buffered main-loop PSUM
        # pool (mps) so each chunk's copy overlaps the next chunk's matmul
        # (pps has bufs=1 and would serialize PE->ACT->PE)
        wc_sb = pre.tile([128, 2, HID], BF16, tag="wc_sb")
        for c in range(2):
            ps_mm = mps.tile([128, 1536], F32, tag="psE", name=f"wc{c}")
            nc.tensor.matmul(ps_mm[:, :HID],
                             wt_sb[:, 128 * c:128 * (c + 1)], w2_sb,
                             start=True, stop=True)
            nc.scalar.copy(out=wc_sb[:, c, :], in_=ps_mm[:, :HID])

        # ---- bias tile DMA issue (sync queue, 6-buffer ring)
        bt_tiles = {}

        def issue_bt(s):
            btT = pbias.tile([128, NDP], F8E5, tag="btT", name=f"bt{s % 8}")
            nc.sync.dma_start(out=btT, in_=bias_t[s])
            bt_tiles[s] = btT

        issue_bt(0)
        issue_bt(1)

        # ---- emb_destT -> gts (= SCALE * Wc.T @ emb_dest.T)
        # dest/src preamble streams ride the ACT DMA queue so they don't
        # serialize behind the bias tiles on the sync queue
        dsb = preb.tile([128, 2, ND], BF16, tag="dsb")
        for c in range(2):
            nc.scalar.dma_start(out=dsb[:, c, :],
                                in_=destT_t[128 * c:128 * (c + 1), :])
        for d0 in range(0, NDP, 512):
            dw = min(512, NDP - d0)
            ps_g = mps.tile([128, 1536], F32, tag="psE", name=f"g{d0}")
            for c in range(2):
                nc.tensor.matmul(ps_g[:, :dw],
                                 wc_sb[:, c, :],
                                 dsb[:, c, d0:d0 + dw],
                                 start=(c == 0), stop=(c == 1))
            nc.scalar.activation(out=gts[:, d0:d0 + dw], in_=ps_g[:, :dw],
                                 func=AF.Copy, scale=SCALE)

        # ---- emb_srcT -> h1T (= W.T @ emb_src.T).
        # Only chunks 0-1 are produced in the preamble; chunks 2-7 are
        # injected into the main loop right before first use so the
        # in-order PE queue doesn't hold e-mm(0) behind 32 h1t matmuls
        # (whose last DMA dependency is the end of the src stream).
        nc.gpsimd.memset(h1t[:, N_SRC:NSP], 0.0)

        def emit_h1_chunk(j, eng):
            j0 = j * HC
            ssb = preb.tile([128, 2, HC], BF16, tag="ssb",
                            name=f"ssb{j % 2}")
            for c in range(2):
                eng.dma_start(
                    out=ssb[:, c, :],
                    in_=srcT_t[128 * c:128 * (c + 1), j0:j0 + HC])
            for half in range(2):
                ps_h = pps.tile([128, 512], F32, tag="ps_b",
                                name=f"psh{j % 2}{half}")
                for c in range(2):
                    nc.tensor.matmul(
                        ps_h[:, :500], w_sb[:, c, :],
                        ssb[:, c, half * 500:half * 500 + 500],
                        start=(c == 0), stop=(c == 1))
                if half == 0:
                    nc.scalar.copy(out=h1t[:, j0:j0 + 500],
                                   in_=ps_h[:, :500])
                else:
                    nc.vector.tensor_copy(out=h1t[:, j0 + 500:j0 + HC],
                                          in_=ps_h[:, :500])

        for j in range(2):
            emit_h1_chunk(j, nc.scalar)
        for s in range(2, 8):
            issue_bt(s)

        psR = pacc.tile([128, 512], F32)  # rows 32k: denom, 32k+1: numer

        H1_TRIG = {5: 2, 13: 3, 21: 4, 29: 5, 37: 6, 45: 7}
        for s in range(NST):
            btT = bt_tiles.pop(s)
            if s in H1_TRIG:
                emit_h1_chunk(H1_TRIG[s], nc.gpsimd)
            if s + 8 < NST:
                issue_bt(s + 8)

            psE = mps.tile([128, 1536], F32, tag="psE")
            for (o, w) in CHK:
                nc.tensor.matmul(psE[:, o:o + w],
                                 h1t[:, 128 * s:128 * (s + 1)],
                                 gts[:, o:o + w], start=True, stop=True)

            # mask fused into the single PSUM-consuming pass:
            # kept (b>0): BIG*b = +huge -> min picks psE
            # masked (b<0): BIG*b = -huge -> -inf in fp16 -> exp() == 0
            sm = pt.tile([128, NDP], FP16, tag="sm")
            nc.vector.scalar_tensor_tensor(
                out=sm, in0=btT, scalar=BIG, in1=psE[:, :NDP],
                op0=OP.mult, op1=OP.min)
            # lrelu split: DVE (mul 4x + max 2x) on cols [:D_DVE], ACT Prelu
            # (parametric_relu lives in the same table set as exp -> no
            # ACT_TABLE_LOAD swaps) on cols [D_DVE:]
            y = pt.tile([128, D_DVE], FP16, tag="y")
            nc.vector.tensor_scalar_mul(y, sm[:, :D_DVE], 0.2)
            t = pt.tile([128, NDP], FP16, tag="t")
            nc.vector.tensor_max(t[:, :D_DVE], sm[:, :D_DVE], y)
            nc.scalar.activation(out=t[:, D_DVE:], in_=sm[:, D_DVE:],
                                 func=AF.Prelu, alpha=0.2)
            u = pu.tile([128, NDP], BF16, tag="u")
            nc.scalar.activation(out=u, in_=t, func=AF.Exp)

            # reduce-mm for the PREVIOUS tile: keeps the in-order PE queue
            # from stalling e-mm(s+1) behind a reduce that waits on exp(s)
            if s > 0:
                up, sp = u_prev
                for k, (o, w) in enumerate(CHK):
                    nc.tensor.matmul(psR[32 * k:32 * k + 2, :w],
                                     ftw[:, 2 * sp:2 * sp + 2],
                                     up[:, o:o + w],
                                     start=(sp == 0), stop=False)
            u_prev = (u, s)

        up, sp = u_prev
        for k, (o, w) in enumerate(CHK):
            nc.tensor.matmul(psR[32 * k:32 * k + 2, :w],
                             ftw[:, 2 * sp:2 * sp + 2], up[:, o:o + w],
                             start=False, stop=True)

        # ---- finals: out = numer / denom on 3 partitions; denom rows 32k /
        # numer rows 32k+1 gathered straight from PSUM by two strided DMAs
        # (DMA descriptors handle partition strides; DVE ops cannot)
        rsb = psm.tile([66, 512], F32, tag="rsb")
        nc.scalar.copy(out=rsb, in_=psR[:66, :])
        d3 = psm.tile([3, 512], F32, tag="d3")
        n3 = psm.tile([3, 512], F32, tag="n3")
        nc.sync.dma_start(out=d3, in_=rsb[0:65:32, :])
        nc.sync.dma_start(out=n3, in_=rsb[1:66:32, :])
        rec3 = psm.tile([3, 512], F32, tag="rec3")
        scr3 = psm.tile([3, 512], F32, tag="scr3")
        nc.vector.reciprocal_approx_accurate(out=rec3, in_=d3, scratch=scr3)
        o3 = psm.tile([3, 512], F32, tag="o3")
        nc.vector.tensor_mul(o3, n3, rec3)
        for k, (o, w) in enumerate(CHK):
            we = min(o + w, ND) - o
            nc.sync.dma_start(out=out_t[:, o:o + we], in_=o3[k:k + 1, :we])

    nc.compile()
    return nc


def _get_nc():
    if "nc" not in _CACHE:
        _CACHE["nc"] = _build_nc()
    return _CACHE["nc"]


def kernel(bias, emb_dest, emb_src, feature_src, W, W2, _trace=False):
    import ml_dtypes
    from concourse.bass_utils import run_bass_kernel_spmd

    BF = ml_dtypes.bfloat16
    F8 = ml_dtypes.float8_e5m2

    bias = np.ascontiguousarray(bias, dtype=np.float32)
    emb_dest = np.ascontiguousarray(emb_dest, dtype=np.float32)
    emb_src = np.ascontiguousarray(emb_src, dtype=np.float32)
    ft = np.ascontiguousarray(feature_src, dtype=np.float32).reshape(-1)
    W = np.ascontiguousarray(W, dtype=np.float32)
    W2 = np.ascontiguousarray(W2, dtype=np.float32)

    nan_ind = np.isnan(ft)
    if nan_ind.any():
        # NaN source features: zero the feature and mask out the column
        # (matches reference semantics). Never hit for randn inputs.
        ft = np.where(nan_ind, 0.0, ft)
        bias = np.where(nan_ind.reshape(1, -1), -1.0, bias)

    srcT = np.ascontiguousarray(emb_src.T.astype(BF))    # [256, 8000] bf16
    ftp = np.zeros(NSP, dtype=np.float32)
    ftp[:N_SRC] = ft
    ft_cols = np.ascontiguousarray(ftp.reshape(NST, 128).T)  # [128, 63]

    # bias staged as fp8e5 (only the sign is consumed on device; e5m2 is
    # sign-exact for |x| >= 2^-17, i.e. all but ~1e-5 of randn mass)
    bias8 = bias.astype(F8)

    Wb = W.astype(BF)
    nc = _get_nc()
    in_maps = []
    for i in range(N_CORES):
        r0 = i * ND
        slabT = np.zeros((NSP, NDP), dtype=F8)
        slabT[:N_SRC, :ND] = bias8[r0:r0 + ND].T
        in_maps.append({
            "biasT": slabT.reshape(NST, 128, NDP),
            "emb_destT": np.ascontiguousarray(emb_dest[r0:r0 + ND].T.astype(BF)),
            "emb_srcT": srcT,
            "ft_cols": ft_cols,
            "W": Wb,
            "WT": np.ascontiguousarray(Wb.T),
            "W2": W2.astype(BF),
        })
    res = run_bass_kernel_spmd(nc, in_maps, list(range(N_CORES)),
                               trace=_trace)
    out = np.concatenate(
        [res.results[i]["out"].reshape(ND, 1) for i in range(N_CORES)], axis=0)
    if _trace:
        return out, res
    return out
